# revision 1
# baseline (speedup 1.0000x reference)
"""BandLinear kernel for 8 TRN2 NeuronCores.

out[n, o] = sum_i x[n, i] * (weight * mask)[o, i] + bias[o]
with a +-8 band mask, x: [16384, 4096] f32.

Strategy (data-parallel over tokens, 2048 tokens/core):
 - Host pre-transposes each core's x shard into in-feature-block tiles
   xt[k, p, n] = x[n, 128k + p]  (k: 32 in-blocks, p: partition, n: token)
 - Weights are packed per out-block j as 3 stationary couplings
   ws[j, p, 128d + m] = (weight*mask)[128j + m, 128(j-1+d) + p]
 - On device, for each out-block j, psum[o_local, n] accumulates 2-3
   matmuls (stationary = coupling weights, moving = x tiles, N=512
   token chunks, one PSUM bank per chunk). Bias is per-partition and is
   fused into the PSUM->SBUF drain (ScalarE Identity / VectorE
   tensor_scalar). Device writes out^T [4096, 2048]; host un-transposes.
"""

import os
import sys

for _p in ("/opt/trn_rl_repo", "/root/.axon_site/_ro/trn_rl_repo"):
    if os.path.isdir(_p) and _p not in sys.path:
        sys.path.append(_p)

import numpy as np
import ml_dtypes

import concourse.bacc as bacc
import concourse.mybir as mybir
from concourse.bass_utils import run_bass_kernel_spmd
from concourse.tile import TileContext

N_CORES = 8
N_TOK = 16384
NF = 4096
BAND = 8
TPC = N_TOK // N_CORES          # tokens per core (2048)
KB = NF // 128                  # 32 feature blocks
CC = TPC // 512                 # token chunks of 512 per core (4)

# compute dtype: "float32r" (1 cyc/row, ~2e-4), "bfloat16" (~3e-3, half DMA),
# "float32" (exact, 4 cyc/row)
COMPUTE_DT = os.environ.get("BAND_COMPUTE_DT", "float32r")
# output storage dtype: "float32" or "bfloat16"
OUT_DT = os.environ.get("BAND_OUT_DT", "float32")

LAST_RESULT = None  # BassKernelResults of the most recent run (for test.py)

_cache = {}


def _np_dt(name):
    return ml_dtypes.bfloat16 if name == "bfloat16" else np.float32


def _build(compute_dt: str, out_dt: str):
    cdt = getattr(mybir.dt, compute_dt)
    odt = getattr(mybir.dt, out_dt)
    f32 = mybir.dt.float32
    nc = bacc.Bacc("TRN2", target_bir_lowering=False, debug=False,
                   num_devices=N_CORES)
    XT = nc.dram_tensor("xt", [KB, 128, TPC], cdt, kind="ExternalInput").ap()
    WS = nc.dram_tensor("ws", [KB, 128, 384], cdt, kind="ExternalInput").ap()
    BM = nc.dram_tensor("bm", [128, KB], f32, kind="ExternalInput").ap()
    OT = nc.dram_tensor("out", [NF, TPC], odt, kind="ExternalOutput").ap()

    ident = mybir.ActivationFunctionType.Identity
    add = mybir.AluOpType.add

    with TileContext(nc) as tc:
        with (
            tc.tile_pool(name="bp", bufs=1) as bp,
            tc.tile_pool(name="xp", bufs=5) as xp,
            tc.tile_pool(name="wp", bufs=3) as wp,
            tc.tile_pool(name="op", bufs=3) as op,
            tc.tile_pool(name="pp", bufs=8, space="PSUM") as pp,
        ):
            bias_sb = bp.tile([128, KB], f32)
            nc.sync.dma_start(out=bias_sb[:], in_=BM[:])

            xk_sb = {}

            def load_xk(k):
                t = xp.tile([128, TPC], cdt, tag="xk")
                nc.sync.dma_start(out=t[:], in_=XT[k])
                xk_sb[k] = t

            load_xk(0)
            load_xk(1)
            for j in range(KB):
                wj = wp.tile([128, 384], cdt, tag="w")
                nc.sync.dma_start(out=wj[:], in_=WS[j])
                if j + 2 < KB:
                    load_xk(j + 2)
                oj = op.tile([128, TPC], odt, tag="o")
                ds = [d for d in range(3) if 0 <= j - 1 + d < KB]
                for c in range(CC):
                    p = pp.tile([128, 512], f32, tag="ps")
                    for i, d in enumerate(ds):
                        nc.tensor.matmul(
                            p[:],
                            wj[:, 128 * d:128 * d + 128],
                            xk_sb[j - 1 + d][:, 512 * c:512 * c + 512],
                            start=(i == 0),
                            stop=(i == len(ds) - 1),
                        )
                    osl = oj[:, 512 * c:512 * c + 512]
                    bsl = bias_sb[:, j:j + 1]
                    if (j + c) % 2 == 0:
                        nc.scalar.activation(osl, p[:], ident, bias=bsl)
                    else:
                        nc.vector.tensor_scalar(osl, p[:], bsl, None, op0=add)
                nc.sync.dma_start(out=OT[128 * j:128 * j + 128, :], in_=oj[:])
    nc.finalize()
    return nc


def _get_nc(compute_dt, out_dt):
    key = (compute_dt, out_dt)
    if key not in _cache:
        _cache[key] = _build(compute_dt, out_dt)
    return _cache[key]


def kernel(x, weight, bias, mask):
    global LAST_RESULT
    x = np.asarray(x, dtype=np.float32)
    weight = np.asarray(weight, dtype=np.float32)
    bias = np.asarray(bias, dtype=np.float32)
    mask = np.asarray(mask, dtype=np.float32)

    cnp = _np_dt(COMPUTE_DT)
    wm = weight * mask                      # [O, I]

    # ws[j, p, 128d + m] = wm[128j + m, 128(j-1+d) + p]
    ws = np.zeros((KB, 128, 384), dtype=np.float32)
    for j in range(KB):
        for d in range(3):
            jj = j - 1 + d
            if 0 <= jj < KB:
                blk = wm[128 * j:128 * j + 128, 128 * jj:128 * jj + 128]
                ws[j, :, 128 * d:128 * d + 128] = blk.T
    ws = np.ascontiguousarray(ws.astype(cnp))

    bm = np.ascontiguousarray(bias.reshape(KB, 128).T.astype(np.float32))

    in_maps = []
    for ci in range(N_CORES):
        xs = x[TPC * ci:TPC * (ci + 1)]               # [TPC, NF]
        xt = np.ascontiguousarray(xs.T.astype(cnp)).reshape(KB, 128, TPC)
        in_maps.append({"xt": xt, "ws": ws, "bm": bm})

    nc = _get_nc(COMPUTE_DT, OUT_DT)
    LAST_RESULT = run_bass_kernel_spmd(nc, in_maps, list(range(N_CORES)))

    out = np.empty((N_TOK, NF), dtype=np.float32)
    for ci in range(N_CORES):
        ot = np.asarray(LAST_RESULT.results[ci]["out"], dtype=np.float32)
        out[TPC * ci:TPC * (ci + 1)] = ot.T
    return out


# revision 3
# speedup vs baseline: 1.7592x; 1.7592x over previous
"""BandLinear kernel for 8 TRN2 NeuronCores.

out[n, o] = sum_i x[n, i] * (weight * mask)[o, i] + bias[o]
with a +-8 band mask, x: [16384, 4096] f32.

Strategy (data-parallel over tokens, 2048 tokens/core):
 - Host pre-transposes each core's x shard into in-feature-block tiles
   xt[k, p, n] = x[n, 128k + p]  (k: 32 in-blocks, p: partition, n: token)
 - Weights are packed per out-block j as 3 stationary couplings
   ws[j, p, 128d + m] = (weight*mask)[128j + m, 128(j-1+d) + p]
 - On device, for each out-block j, psum[o_local, n] accumulates 2-3
   matmuls (stationary = coupling weights, moving = x tiles, N=512
   token chunks, one PSUM bank per chunk). Bias is per-partition and is
   fused into the PSUM->SBUF drain (ScalarE Identity / VectorE
   tensor_scalar). Device writes out^T [4096, 2048]; host un-transposes.
"""

import os
import sys

for _p in ("/opt/trn_rl_repo", "/root/.axon_site/_ro/trn_rl_repo"):
    if os.path.isdir(_p) and _p not in sys.path:
        sys.path.append(_p)

import numpy as np
import ml_dtypes

import concourse.bacc as bacc
import concourse.mybir as mybir
from concourse.bass_utils import run_bass_kernel_spmd
from concourse.tile import TileContext

N_CORES = 8
N_TOK = 16384
NF = 4096
BAND = 8
TPC = N_TOK // N_CORES          # tokens per core (2048)
KB = NF // 128                  # 32 feature blocks
CC = TPC // 512                 # token chunks of 512 per core (4)

# compute dtype: "float32r" (1 cyc/row, ~2e-4), "bfloat16" (~3e-3, half DMA),
# "float32" (exact, 4 cyc/row)
COMPUTE_DT = os.environ.get("BAND_COMPUTE_DT", "float32r")
# output storage dtype: "float32" or "bfloat16"
OUT_DT = os.environ.get("BAND_OUT_DT", "float32")

LAST_RESULT = None  # BassKernelResults of the most recent run (for test.py)

_cache = {}


def _np_dt(name):
    return ml_dtypes.bfloat16 if name == "bfloat16" else np.float32


def _build(compute_dt: str, out_dt: str):
    cdt = getattr(mybir.dt, compute_dt)
    odt = getattr(mybir.dt, out_dt)
    f32 = mybir.dt.float32
    nc = bacc.Bacc("TRN2", target_bir_lowering=False, debug=False,
                   num_devices=N_CORES)
    XT = nc.dram_tensor("xt", [KB, 128, TPC], cdt, kind="ExternalInput").ap()
    WS = nc.dram_tensor("ws", [4, 128, (KB // 4) * 384], cdt,
                        kind="ExternalInput").ap()
    BM = nc.dram_tensor("bm", [128, KB], f32, kind="ExternalInput").ap()
    OT = nc.dram_tensor("out", [NF, TPC], odt, kind="ExternalOutput").ap()

    ident = mybir.ActivationFunctionType.Identity
    add = mybir.AluOpType.add
    WCH = (KB // 4) * 384          # ws chunk width (8 j's worth)

    with TileContext(nc) as tc:
        with (
            tc.tile_pool(name="bp", bufs=1) as bp,
            tc.tile_pool(name="xp", bufs=8) as xp,
            tc.tile_pool(name="op", bufs=4) as op,
            tc.tile_pool(name="pp", bufs=8, space="PSUM") as pp,
        ):
            bias_sb = bp.tile([128, KB], f32)
            nc.sync.dma_start(out=bias_sb[:], in_=BM[:])
            ws_sb = bp.tile([128, KB * 384], cdt)

            xk_sb = {}

            def load_xk(k):
                t = xp.tile([128, TPC], cdt, tag="xk")
                nc.sync.dma_start(out=t[:], in_=XT[k])
                xk_sb[k] = t

            # Interleave initial x prefetches with the 4 weight chunks so
            # the first matmuls start early while DMA stays saturated.
            load_xk(0)
            load_xk(1)
            nc.sync.dma_start(out=ws_sb[:, 0:WCH], in_=WS[0])
            load_xk(2)
            load_xk(3)
            nc.sync.dma_start(out=ws_sb[:, WCH:2 * WCH], in_=WS[1])
            load_xk(4)
            nc.sync.dma_start(out=ws_sb[:, 2 * WCH:3 * WCH], in_=WS[2])
            load_xk(5)
            nc.sync.dma_start(out=ws_sb[:, 3 * WCH:4 * WCH], in_=WS[3])

            for j in range(KB):
                if j + 6 < KB:
                    load_xk(j + 6)
                oj = op.tile([128, TPC], odt, tag="o")
                ds = [d for d in range(3) if 0 <= j - 1 + d < KB]
                for c in range(CC):
                    p = pp.tile([128, 512], f32, tag="ps")
                    for i, d in enumerate(ds):
                        nc.tensor.matmul(
                            p[:],
                            ws_sb[:, j * 384 + 128 * d:j * 384 + 128 * d + 128],
                            xk_sb[j - 1 + d][:, 512 * c:512 * c + 512],
                            start=(i == 0),
                            stop=(i == len(ds) - 1),
                        )
                    osl = oj[:, 512 * c:512 * c + 512]
                    bsl = bias_sb[:, j:j + 1]
                    if (j + c) % 2 == 0:
                        nc.scalar.activation(osl, p[:], ident, bias=bsl)
                    else:
                        nc.vector.tensor_scalar(osl, p[:], bsl, None, op0=add)
                nc.sync.dma_start(out=OT[128 * j:128 * j + 128, :], in_=oj[:])
    nc.finalize()
    return nc


def _get_nc(compute_dt, out_dt):
    key = (compute_dt, out_dt)
    if key not in _cache:
        _cache[key] = _build(compute_dt, out_dt)
    return _cache[key]


def kernel(x, weight, bias, mask):
    global LAST_RESULT
    x = np.asarray(x, dtype=np.float32)
    weight = np.asarray(weight, dtype=np.float32)
    bias = np.asarray(bias, dtype=np.float32)
    mask = np.asarray(mask, dtype=np.float32)

    cnp = _np_dt(COMPUTE_DT)
    wm = weight * mask                      # [O, I]

    # ws[j, p, 128d + m] = wm[128j + m, 128(j-1+d) + p]
    ws = np.zeros((KB, 128, 384), dtype=np.float32)
    for j in range(KB):
        for d in range(3):
            jj = j - 1 + d
            if 0 <= jj < KB:
                blk = wm[128 * j:128 * j + 128, 128 * jj:128 * jj + 128]
                ws[j, :, 128 * d:128 * d + 128] = blk.T
    ws = np.ascontiguousarray(
        ws.astype(cnp).reshape(4, KB // 4, 128, 384).transpose(0, 2, 1, 3)
        .reshape(4, 128, (KB // 4) * 384))

    bm = np.ascontiguousarray(bias.reshape(KB, 128).T.astype(np.float32))

    in_maps = []
    for ci in range(N_CORES):
        xs = x[TPC * ci:TPC * (ci + 1)]               # [TPC, NF]
        xt = np.ascontiguousarray(xs.T.astype(cnp)).reshape(KB, 128, TPC)
        in_maps.append({"xt": xt, "ws": ws, "bm": bm})

    nc = _get_nc(COMPUTE_DT, OUT_DT)
    LAST_RESULT = run_bass_kernel_spmd(nc, in_maps, list(range(N_CORES)))

    out = np.empty((N_TOK, NF), dtype=np.float32)
    for ci in range(N_CORES):
        ot = np.asarray(LAST_RESULT.results[ci]["out"], dtype=np.float32)
        out[TPC * ci:TPC * (ci + 1)] = ot.T
    return out
